# revision 3
# baseline (speedup 1.0000x reference)
"""CenterContrastiveLoss forward on 8 Trainium2 NeuronCores — v2.

loss = mean_i ||e_i - c_{y_i}||^2 + mean_i min_j( d_ij * (1 - onehot) )
with d_ij = ||e_i||^2 + ||c_j||^2 - 2 e_i.c_j.

Data-parallel over batch (2048 rows/core), centers replicated.

v2 changes vs baseline:
  - GEMM in fp8(e4m3) DoubleRow perf mode: operands host-cast to fp8 in
    [128, ko, x] k-subtile layout; 2 MMs per 512-wide chunk instead of 4.
  - PSUM evacuation split: ScalarE copies PSUM->SBUF bf16, DVE does the
    (clamp0 + min-reduce) at 4x mode from SBUF (vs 1x from PSUM).
  - csq row injected via K=1 matmuls placed as the *stop* MM of each
    accumulation group (prep no longer gates GEMM start), optionally
    packed 4-wide with tile_position row groups.
  - all inputs host-cast to bf16/fp8: HBM traffic 20MB -> ~11MB/core.
Host combines the 8 x [128, 2] partial sums.
"""

import numpy as np

import concourse.bass as bass
import concourse.tile as tile
from concourse import mybir
from concourse.bass_utils import run_bass_kernel_spmd

NCORES = 8
B, D, C = 16384, 512, 4096
BS = B // NCORES  # 2048 rows per core
P = 128
KO = D // P  # 4 k-subtiles
BT = BS // P  # 16 batch tiles per core
CH = 4  # psum chunks over classes (each [128, 1024] = 2 banks)
CHW = C // CH  # 1024
NSL = CHW // 512  # 2 matmul N-slices per chunk

F32 = mybir.dt.float32
BF16 = mybir.dt.bfloat16
I32 = mybir.dt.int32
FP8 = mybir.dt.float8e4

# chosen via on-device benchmarks: dr8 + act+dve4 + inject packing
GEMM = "dr8"  # "dr8" | "bf16"
EVAC = "act+dve4"  # "act+dve4" | "dve_fused" | "split"
INJECT_PACK = True


def _split_excess_waits(nc, cap=1):
    # This walrus build encodes at most one sync-wait per instruction, but
    # TileContext's wait assignment can attach several. Hoist the excess
    # onto same-engine NoOps inserted just before the instruction.
    counter = 0
    for f in nc.m.functions:
        for blk in f.blocks:
            insts = list(blk.instructions)
            if not any(
                i.sync_info is not None
                and i.sync_info.on_wait
                and len(i.sync_info.on_wait) > cap
                for i in insts
            ):
                continue
            out = []
            for inst in insts:
                si = inst.sync_info
                waits = list(si.on_wait) if si is not None and si.on_wait else []
                if len(waits) > cap:
                    extra, keep = waits[:-cap], waits[-cap:]
                    for j in range(0, len(extra), cap):
                        counter += 1
                        nop = mybir.InstNoOp(name=f"I-wsplit-{counter}")
                        nop.engine = inst.engine
                        nop.sync_info = mybir.SyncInfo(
                            on_wait=list(extra[j : j + cap]), on_update=[]
                        )
                        out.append(nop)
                    si.on_wait = keep
                out.append(inst)
            blk.instructions = out
    return nc


def _build(bench_iters=None, gemm=GEMM, evac=EVAC, inject_pack=INJECT_PACK):
    GDT = FP8 if gemm == "dr8" else BF16
    nc = bass.Bass()
    etp = nc.dram_tensor("etp", [P, KO, BS], GDT, kind="ExternalInput")
    ctp = nc.dram_tensor("ctp", [P, KO, C], GDT, kind="ExternalInput")
    sqc = nc.dram_tensor("sqc", [P, KO, C], BF16, kind="ExternalInput")
    emb = nc.dram_tensor("emb", [BS, D], BF16, kind="ExternalInput")
    cen = nc.dram_tensor("cen", [C, D], BF16, kind="ExternalInput")
    labels = nc.dram_tensor("labels", [BS], I32, kind="ExternalInput")
    partials = nc.dram_tensor("partials", [P, 2], F32, kind="ExternalOutput")

    with tile.TileContext(nc) as tc:
        with (
            tc.tile_pool(name="const", bufs=1) as const_pool,
            tc.tile_pool(name="big", bufs=1) as big_pool,
            tc.tile_pool(name="work", bufs=3) as work_pool,
            tc.tile_pool(name="stage", bufs=3) as stage_pool,
            tc.tile_pool(name="acc", bufs=1) as acc_pool,
            tc.tile_pool(name="pg", bufs=4, space="PSUM") as pg,
        ):
            ones4 = const_pool.tile([P, P], BF16, tag="ones4")
            nc.gpsimd.memset(ones4[:], 1.0)
            # indicator rows for csq replication to partitions {0,32,64,96}
            ind4 = const_pool.tile([P, P], BF16, tag="ind4")
            nc.gpsimd.memset(ind4[:], 0.0)
            csq4 = const_pool.tile([P, C], BF16, tag="csq4")

            et_sb = big_pool.tile([P, KO, BS], GDT, tag="et_sb")
            ct_sb = big_pool.tile([P, KO, C], GDT, tag="ct_sb")
            sqc_sb = big_pool.tile([P, KO, C], BF16, tag="sqc_sb")
            for h in range(2):
                ksl = slice(2 * h, 2 * h + 2)
                nc.sync.dma_start(et_sb[:, ksl, :], etp[:, ksl, :])
                nc.sync.dma_start(ct_sb[:, ksl, :], ctp[:, ksl, :])
                nc.sync.dma_start(sqc_sb[:, ksl, :], sqc[:, ksl, :])

            e_sq = acc_pool.tile([P, BT], F32, tag="e_sq")
            possum = acc_pool.tile([P, BT], F32, tag="possum")
            negrow = acc_pool.tile([P, BT], F32, tag="negrow")
            out_sb = acc_pool.tile([P, 2], F32, tag="out_sb")

            g_all = None
            if bench_iters is not None:
                # indirect DMA is not encodable inside For_i on this
                # toolchain: pre-gather centers[labels] outside the loop.
                g_all = big_pool.tile([P, BT, D], BF16, tag="g_all")
                for bt in range(BT):
                    bsl = slice(bt * P, (bt + 1) * P)
                    lab = work_pool.tile([P, 1], I32, tag="lab")
                    nc.sync.dma_start(lab[:], labels[bsl, None])
                    nc.gpsimd.indirect_dma_start(
                        out=g_all[:, bt, :],
                        out_offset=None,
                        in_=cen[:],
                        in_offset=bass.IndirectOffsetOnAxis(ap=lab[:, :1], axis=0),
                    )
                loop_cm = tc.For_i(0, bench_iters, 1)
                loop_cm.__enter__()

            # ---- set up ind4: 1.0 at cols {0,32,64,96} of row 0 ----
            # (memset-based: write 1.0 into 4 single elements)
            for i in range(4 if inject_pack else 1):
                nc.gpsimd.memset(ind4[0:1, 32 * i : 32 * i + 1], 1.0)

            # ---- csq4[j] at partitions {0,32,64,96}: ones-matmul over sqc,
            # then an indicator matmul to replicate across row groups ----
            for ch8 in range(8):
                csl = slice(ch8 * 512, (ch8 + 1) * 512)
                ps_a = pg.tile([P, CHW], F32, tag="pgemm", name=f"pcsq_{ch8}")
                for ko in range(KO):
                    nc.tensor.matmul(
                        ps_a[0:1, 0:512],
                        lhsT=ones4[:, 0:1],
                        rhs=sqc_sb[:, ko, csl],
                        start=(ko == 0),
                        stop=(ko == KO - 1),
                    )
                ctmp = work_pool.tile([1, 512], BF16, tag="ctmp")
                nc.scalar.mul(ctmp[:], ps_a[0:1, 0:512], 0.25)
                ps_b = pg.tile([P, CHW], F32, tag="pgemm", name=f"pcsqb_{ch8}")
                nc.tensor.matmul(
                    ps_b[:, 0:512],
                    lhsT=ind4[0:1, :],
                    rhs=ctmp[:],
                    start=True,
                    stop=True,
                )
                nc.scalar.copy(csq4[:, csl], ps_b[:, 0:512])

            # ---- positive term + ||e||^2 per batch tile ----
            for bt in range(BT):
                bsl = slice(bt * P, (bt + 1) * P)
                if g_all is None:
                    lab = work_pool.tile([P, 1], I32, tag="lab")
                    nc.sync.dma_start(lab[:], labels[bsl, None])
                    g = work_pool.tile([P, D], BF16, tag="g")
                    nc.gpsimd.indirect_dma_start(
                        out=g[:],
                        out_offset=None,
                        in_=cen[:],
                        in_offset=bass.IndirectOffsetOnAxis(ap=lab[:, :1], axis=0),
                    )
                else:
                    g = g_all[:, bt, :]
                e = work_pool.tile([P, D], BF16, tag="e")
                nc.sync.dma_start(e[:], emb[bsl, :])

                esq_scr = stage_pool.tile([P, D], BF16, tag="esq_scr")
                nc.scalar.activation(
                    esq_scr[:],
                    e[:],
                    mybir.ActivationFunctionType.Square,
                    accum_out=e_sq[:, bt : bt + 1],
                )
                diff = stage_pool.tile([P, D], BF16, tag="diff")
                nc.vector.tensor_sub(diff[:], e[:], g[:])
                psq_scr = stage_pool.tile([P, D], BF16, tag="psq_scr")
                nc.scalar.activation(
                    psq_scr[:],
                    diff[:],
                    mybir.ActivationFunctionType.Square,
                    accum_out=possum[:, bt : bt + 1],
                )

            # ---- main GEMM loop over batch tiles ----
            for bt in range(BT):
                bsl = slice(bt * P, (bt + 1) * P)
                pss = [
                    pg.tile([P, CHW], F32, tag="pgemm", name=f"pg_{bt}_{i}")
                    for i in range(CH)
                ]
                # DR (or bf16) accumulation MMs, one group per 512-slice.
                # Chunk-pair blocking: k-group outer within a 2-chunk block
                # (4 LDWEIGHTS per bt, not 16) while evacuation of one block
                # still overlaps compute of the next.
                ngrp = 2 if gemm == "dr8" else KO
                for cb in range(CH // 2):
                    chs = (2 * cb, 2 * cb + 1)
                    for gp in range(ngrp):
                        for ch in chs:
                            for s in range(NSL):
                                osl = slice(s * 512, (s + 1) * 512)
                                csl = slice(
                                    ch * CHW + s * 512, ch * CHW + (s + 1) * 512
                                )
                                if gemm == "dr8":
                                    nc.tensor.matmul(
                                        pss[ch][:, osl],
                                        lhsT=et_sb[:, 2 * gp : 2 * gp + 2, bsl],
                                        rhs=ct_sb[:, 2 * gp : 2 * gp + 2, csl],
                                        start=(gp == 0),
                                        stop=False,
                                        perf_mode=mybir.MatmulPerfMode.DoubleRow,
                                    )
                                else:
                                    nc.tensor.matmul(
                                        pss[ch][:, osl],
                                        lhsT=et_sb[:, gp, bsl],
                                        rhs=ct_sb[:, gp, csl],
                                        start=(gp == 0),
                                        stop=False,
                                    )
                    # csq injection MMs (stop=True closes each group), issued
                    # consecutively so row-group packing can overlap them
                    for ch in chs:
                        for s in range(NSL):
                            osl = slice(s * 512, (s + 1) * 512)
                            csl = slice(
                                ch * CHW + s * 512, ch * CHW + (s + 1) * 512
                            )
                            i4 = (ch * NSL + s) % 4
                            rg = 32 * i4 if inject_pack else 0
                            nc.tensor.matmul(
                                pss[ch][:, osl],
                                lhsT=ones4[rg : rg + 1, :],
                                rhs=csq4[rg : rg + 1, csl],
                                start=False,
                                stop=True,
                                tile_position=(rg, 0) if inject_pack else None,
                            )

                # evacuation + min-reduce
                cmins = stage_pool.tile([P, CH], F32, tag="cmins")
                for ch in range(CH):
                    mode = evac
                    if evac == "split":
                        mode = "dve_fused" if ch < 2 else "act+dve4"
                    if mode == "act+dve4":
                        st = stage_pool.tile([P, CHW], BF16, tag="st")
                        nc.scalar.copy(st[:], pss[ch][:])
                        ms = stage_pool.tile([P, CHW], BF16, tag="ms")
                        nc.vector.tensor_scalar(
                            ms[:],
                            st[:],
                            0.0,
                            0.0,
                            mybir.AluOpType.add,
                            mybir.AluOpType.min,
                            accum_out=cmins[:, ch : ch + 1],
                        )
                    else:
                        ms = stage_pool.tile([P, CHW], BF16, tag="ms")
                        nc.vector.tensor_scalar(
                            ms[:],
                            pss[ch][:],
                            0.0,
                            0.0,
                            mybir.AluOpType.add,
                            mybir.AluOpType.min,
                            accum_out=cmins[:, ch : ch + 1],
                        )
                nc.vector.tensor_reduce(
                    negrow[:, bt : bt + 1],
                    cmins[:],
                    op=mybir.AluOpType.min,
                    axis=mybir.AxisListType.X,
                )

            # ---- final per-partition sums ----
            # negfin = min(negrow + e_sq, 0): the reference's (1 - onehot)
            # mask makes the label entry exactly 0, so each row-min is
            # min(0, min_j d). (cmins already clamped at 0; equivalent.)
            negadd = acc_pool.tile([P, BT], F32, tag="negadd")
            nc.vector.tensor_add(negadd[:], negrow[:], e_sq[:])
            negfin = acc_pool.tile([P, BT], F32, tag="negfin")
            nc.vector.tensor_scalar(
                negfin[:], negadd[:], 0.0, None, mybir.AluOpType.min
            )
            nc.vector.reduce_sum(out_sb[:, 0:1], possum[:], axis=mybir.AxisListType.X)
            nc.vector.reduce_sum(out_sb[:, 1:2], negfin[:], axis=mybir.AxisListType.X)

            if bench_iters is not None:
                loop_cm.__exit__(None, None, None)
            nc.sync.dma_start(partials[:], out_sb[:])

    return _split_excess_waits(nc)


_NC_CACHE = None


def _get_nc():
    global _NC_CACHE
    if _NC_CACHE is None:
        _NC_CACHE = _build()
    return _NC_CACHE


def _prep_core(emb_f32, lab_i32, ctp8, sqc16, cen16):
    import ml_dtypes

    GDT8 = ml_dtypes.float8_e4m3 if GEMM == "dr8" else ml_dtypes.bfloat16
    # [128, KO, BS] k-subtile layout of emb^T
    et = np.ascontiguousarray(
        emb_f32.T.reshape(KO, P, BS).transpose(1, 0, 2)
    ).astype(GDT8)
    return {
        "etp": et,
        "ctp": ctp8,
        "sqc": sqc16,
        "emb": emb_f32.astype(ml_dtypes.bfloat16),
        "cen": cen16,
        "labels": lab_i32,
    }


def make_in_maps(inputs):
    import ml_dtypes

    emb_f = np.ascontiguousarray(np.asarray(inputs["embeddings"], dtype=np.float32))
    lab = np.asarray(inputs["labels"]).astype(np.int32)
    cen_f = np.ascontiguousarray(np.asarray(inputs["centers"], dtype=np.float32))
    assert emb_f.shape == (B, D) and cen_f.shape == (C, D) and lab.shape == (B,)

    GDT8 = ml_dtypes.float8_e4m3 if GEMM == "dr8" else ml_dtypes.bfloat16
    cT = cen_f.T  # [D, C]
    ctp8 = np.ascontiguousarray(
        (-2.0 * cT).reshape(KO, P, C).transpose(1, 0, 2)
    ).astype(GDT8)
    sqc16 = np.ascontiguousarray(
        (4.0 * cT * cT).reshape(KO, P, C).transpose(1, 0, 2)
    ).astype(ml_dtypes.bfloat16)
    cen16 = cen_f.astype(ml_dtypes.bfloat16)

    in_maps = []
    for c in range(NCORES):
        sl = slice(c * BS, (c + 1) * BS)
        in_maps.append(_prep_core(emb_f[sl], lab[sl], ctp8, sqc16, cen16))
    return in_maps


def finalize(res):
    total = 0.0
    for r in res:
        total += float(r["partials"].astype(np.float64).sum())
    return np.float32(total / B)


def kernel(embeddings, labels, centers):
    in_maps = make_in_maps(
        {"embeddings": embeddings, "labels": labels, "centers": centers}
    )
    nc = _get_nc()
    res = run_bass_kernel_spmd(nc, in_maps, list(range(NCORES))).results
    return finalize(res)
